# revision 1
# baseline (speedup 1.0000x reference)
"""Gated-attention (Qwen-style) Trainium2 kernel — bf16 edition.

Sharding (8 cores): data-parallel over batch (2) x tensor-parallel over head
groups (4). Core c handles batch b=c//4 and head group g=c%4: q heads
4g..4g+3, kv heads 2g..2g+1, gate logits 4g..4g+3, w_o columns 512g..512g+512.
Each core computes a partial output y_g = attn_out_g @ w_o[:, cols_g].T in
bf16; the host sums the 4 partials per batch in f32.

Device-side layout: qkv projection computed with head-rows on the M axis in
bf16 (FWL fast weight load), so qT/kT come out as [d, tokens] (ready for the
QK^T matmul) and v is PE-transposed (bf16, 1 cyc/row) to [tokens, d]. Gate
weights ride in the same packed weight tensor (cols 1024:1028). Softmax is
computed in transposed [k, q] layout: exp on ACT straight from PSUM to bf16.
The denominator is accumulated on DVE (16 adds per head-block) + one
ones-matmul; the per-token scale sigmoid(gate)/denom is broadcast across
partitions with a K=1 PE matmul (213ns) instead of gpsimd (4.3us).
Out-projection is interleaved per i-block so PE absorbs ACT-bound stretches.
All DMA triggers go through the otherwise-idle GpSimd (Pool) sequencer.
"""

import os
from contextlib import ExitStack

import numpy as np

B, S, HID = 2, 2048, 2048
NH, NKV, HD = 16, 8, 128
GATE = NH
KV_DIM = NKV * HD

N_CORES = 8
TPG = 4            # tensor-parallel group size (head groups)
QH = NH // TPG     # q heads per core = 4
KVH = NKV // TPG   # kv heads per core = 2
IB = 512           # phase-1 token block
NB = S // IB       # 4 blocks
JT = S // 128      # 16 key tiles
IBLK = 512         # phase-2 query block
NI = S // IBLK     # 4 query blocks
WCOL = 1032        # packed qkv+gate weight cols (1028 used, padded)
SCALE = 1.0 / float(np.sqrt(HD))

_CACHE = {}

LAST_EXEC_NS = None
LAST_RESULTS = None


def _build_program():
    import concourse.bass as bass
    import concourse.mybir as mybir
    from concourse import bacc
    from concourse.tile import TileContext

    F32 = mybir.dt.float32
    F32R = mybir.dt.float32r
    F16 = mybir.dt.float16
    AF = mybir.ActivationFunctionType

    nc = bacc.Bacc()

    xT_d = nc.dram_tensor("xT", [HID, S], F16, kind="ExternalInput")
    wqkvT_d = nc.dram_tensor("wqkvT", [HID, WCOL], F16, kind="ExternalInput")
    woT_d = nc.dram_tensor("woT", [QH * HD, HID], F16, kind="ExternalInput")
    cosT_d = nc.dram_tensor("cosT", [HD, S], F32, kind="ExternalInput")
    sinT_d = nc.dram_tensor("sinT", [HD, S], F32, kind="ExternalInput")
    rotm_d = nc.dram_tensor("rotm", [HD, HD], F16, kind="ExternalInput")
    ident_d = nc.dram_tensor("ident", [128, 128], F16, kind="ExternalInput")
    ones1_d = nc.dram_tensor("ones1", [128, 1], F16, kind="ExternalInput")
    onesr_d = nc.dram_tensor("onesr", [1, 128], F32R, kind="ExternalInput")
    y_d = nc.dram_tensor("y", [S, HID], F16, kind="ExternalOutput")

    with TileContext(nc) as tc, ExitStack() as persist:
        const = persist.enter_context(tc.tile_pool(name="const", bufs=1))
        rotm_sb = const.tile([HD, HD], F16, tag="rotm", name="rotm")
        nc.scalar.dma_start(out=rotm_sb, in_=rotm_d[:, :])
        ident_sb = const.tile([128, 128], F16, tag="ident", name="ident")
        nc.scalar.dma_start(out=ident_sb, in_=ident_d[:, :])
        ones1_sb = const.tile([128, 1], F16, tag="ones1", name="ones1")
        nc.scalar.dma_start(out=ones1_sb, in_=ones1_d[:, :])
        onesr_sb = const.tile([1, 128], F32R, tag="onesr", name="onesr")
        nc.scalar.dma_start(out=onesr_sb, in_=onesr_d[:, :])

        # weights on ACT/DVE sequencers so x loads own the SP/Pool DGEs
        wpool = persist.enter_context(tc.tile_pool(name="w", bufs=1))
        wsb = [wpool.tile([128, WCOL], F16, tag=f"w{h}", name=f"w{h}") for h in range(16)]
        for h in range(16):
            nc.scalar.dma_start(out=wsb[h], in_=wqkvT_d[128 * h:128 * (h + 1), :])
        cos_sb = const.tile([HD, S], F32, tag="cos", name="cos")
        nc.scalar.dma_start(out=cos_sb, in_=cosT_d[:, :])
        sin_sb = const.tile([HD, S], F32, tag="sin", name="sin")
        nc.scalar.dma_start(out=sin_sb, in_=sinT_d[:, :])
        wopool = persist.enter_context(tc.tile_pool(name="wo", bufs=1))
        wo_sb = [wopool.tile([128, HID], F16, tag=f"wo{i}", name=f"wo{i}") for i in range(4)]

        qk_pool = persist.enter_context(tc.tile_pool(name="qk", bufs=1))
        qk_sb = [qk_pool.tile([128, S], F16, tag=f"qk{r}", name=f"qk{r}") for r in range(QH + KVH)]
        v_pool = persist.enter_context(tc.tile_pool(name="v", bufs=1))
        v_sb = [v_pool.tile([128, KVH * HD], F16, tag=f"v{t}", name=f"v{t}") for t in range(JT)]
        g_pool = persist.enter_context(tc.tile_pool(name="g", bufs=1))
        sgflat = g_pool.tile([1, QH * S], F32, tag="sgflat", name="sgflat")

        # ---------------- phase 1: qkv projection + rope + v transpose -----
        with ExitStack() as ph1:
            xpool = ph1.enter_context(tc.tile_pool(name="x", bufs=32))
            tmppool = ph1.enter_context(tc.tile_pool(name="tmp", bufs=2))
            vrawpool = ph1.enter_context(tc.tile_pool(name="vraw", bufs=2))
            sgpool = ph1.enter_context(tc.tile_pool(name="sg", bufs=1))

            ps_acc = ph1.enter_context(tc.tile_pool(name="acc", bufs=4, space="PSUM"))
            ps_rot = ph1.enter_context(tc.tile_pool(name="rot", bufs=1, space="PSUM"))
            ps_tp = ph1.enter_context(tc.tile_pool(name="tp", bufs=2, space="PSUM"))
            ps_g = ph1.enter_context(tc.tile_pool(name="psg", bufs=1, space="PSUM"))

            for ib in range(NB):
                sl = slice(IB * ib, IB * (ib + 1))
                xb = []
                for h in range(16):
                    xt = xpool.tile([128, IB], F16, tag="x", name="x")
                    eng = nc.gpsimd if h % 2 == 0 else nc.sync
                    eng.dma_start(out=xt, in_=xT_d[128 * h:128 * (h + 1), sl])
                    xb.append(xt)

                # gate logits (stationary = packed cols 1024:1028)
                psg = ps_g.tile([QH, IB], F32, tag="psg", name="psg")
                for h in range(16):
                    nc.tensor.matmul(psg, wsb[h][:, 1024:1024 + QH], xb[h],
                                     start=(h == 0), stop=(h == 15))
                # sigmoid per block, flattened to partition 0 for phase 2
                eT = sgpool.tile([QH, IB], F32, tag="eT", name="eT")
                nc.scalar.activation(out=eT, in_=psg, func=AF.Exp, scale=-1.0)
                nc.vector.tensor_scalar_add(eT, eT, 1.0)
                sgT = sgpool.tile([QH, IB], F32, tag="sgT", name="sgT")
                nc.vector.reciprocal_approx_fast(out=sgT, in_=eT)
                for h in range(QH):
                    nc.sync.dma_start(out=sgflat[0:1, S * h + IB * ib:S * h + IB * (ib + 1)],
                                      in_=sgT[h:h + 1, :])

                for rg in range(2):
                    accs = [ps_acc.tile([128, IB], F32, tag="acc", name="acc") for _ in range(4)]
                    for h in range(16):
                        for r4 in range(4):
                            r = 4 * rg + r4
                            nc.tensor.matmul(
                                accs[r4], wsb[h][:, 128 * r:128 * (r + 1)], xb[h],
                                start=(h == 0), stop=(h == 15))
                    for r4 in range(4):
                        r = 4 * rg + r4
                        if r < QH + KVH:  # q or k row-tile: apply rope
                            craw = tmppool.tile([128, IB], F16, tag="craw", name="craw")
                            nc.vector.tensor_copy(craw, accs[r4])
                            rps = ps_rot.tile([128, IB], F32, tag="rot", name="rot")
                            nc.tensor.matmul(rps, rotm_sb, craw, start=True, stop=True)
                            t1 = tmppool.tile([128, IB], F32R, tag="t1", name="t1")
                            nc.vector.tensor_mul(t1, accs[r4], cos_sb[:, sl])
                            t2 = tmppool.tile([128, IB], F32R, tag="t2", name="t2")
                            nc.vector.tensor_mul(t2, rps, sin_sb[:, sl])
                            nc.vector.tensor_add(qk_sb[r][:, sl], t1, t2)
                        else:  # v row-tile: transpose to [tokens, d] in bf16
                            vraw = vrawpool.tile([128, IB], F16, tag="vraw", name="vraw")
                            nc.vector.tensor_copy(vraw, accs[r4])
                            vh = r - (QH + KVH)
                            for s2 in range(IB // 128):
                                tp = ps_tp.tile([128, 128], F16, tag="tp", name="tp")
                                nc.tensor.transpose(
                                    tp, vraw[:, 128 * s2:128 * (s2 + 1)], ident_sb)
                                tt = (IB // 128) * ib + s2
                                nc.vector.tensor_copy(
                                    v_sb[tt][:, 128 * vh:128 * (vh + 1)], tp)

            for cc in range(4):
                nc.gpsimd.dma_start(out=wo_sb[cc], in_=woT_d[128 * cc:128 * (cc + 1), :])

        # ---------------- phase 2: attention + gate + out-projection -------
        with ExitStack() as ph2:
            oc_pool = ph2.enter_context(tc.tile_pool(name="oc", bufs=1))
            OC = [oc_pool.tile([128, S], F16, tag=f"oc{h}", name=f"oc{h}") for h in range(QH)]
            epool = ph2.enter_context(tc.tile_pool(name="e", bufs=4))
            accpool = ph2.enter_context(tc.tile_pool(name="dacc", bufs=4))
            scpool = ph2.enter_context(tc.tile_pool(name="sc", bufs=2))
            ypool = ph2.enter_context(tc.tile_pool(name="y", bufs=2))

            ps_s = ph2.enter_context(tc.tile_pool(name="pss", bufs=2, space="PSUM"))
            ps_o = ph2.enter_context(tc.tile_pool(name="pso", bufs=3, space="PSUM"))
            ps_bc = ph2.enter_context(tc.tile_pool(name="psbc", bufs=1, space="PSUM"))
            ps_y = ph2.enter_context(tc.tile_pool(name="psy", bufs=2, space="PSUM"))

            for i in range(NI):
                isl = slice(IBLK * i, IBLK * (i + 1))
                for kv in range(KVH):
                    psos = []
                    accs2 = []
                    for hh in range(2):
                        psos.append(ps_o.tile([128, IBLK], F32, tag="pso", name="pso"))
                        accs2.append(accpool.tile([128, IBLK], F16, tag="dacc", name="dacc"))
                    for j in range(JT):
                        jsl = slice(128 * j, 128 * (j + 1))
                        psss = []
                        for hh in range(2):
                            h = 2 * kv + hh
                            pss = ps_s.tile([128, IBLK], F32, tag="pss", name="pss")
                            nc.tensor.matmul(pss, qk_sb[QH + kv][:, jsl],
                                             qk_sb[h][:, isl], start=True, stop=True)
                            psss.append(pss)
                        es = []
                        for hh in range(2):
                            e = epool.tile([128, IBLK], F16, tag="e", name="e")
                            nc.scalar.activation(out=e, in_=psss[hh], func=AF.Exp,
                                                 scale=SCALE)
                            es.append(e)
                        for hh in range(2):
                            if j == 0:
                                nc.vector.tensor_copy(accs2[hh], es[hh])
                            else:
                                nc.vector.tensor_add(accs2[hh], accs2[hh], es[hh])
                            nc.tensor.matmul(psos[hh],
                                             v_sb[j][:, 128 * kv:128 * (kv + 1)], es[hh],
                                             start=(j == 0), stop=(j == JT - 1))
                    for hh in range(2):
                        h = 2 * kv + hh
                        den = ps_bc.tile([1, IBLK], F32, tag="bc", name="bc")
                        nc.tensor.matmul(den, ones1_sb, accs2[hh],
                                         start=True, stop=True)
                        rec = scpool.tile([1, IBLK], F32, tag="rec", name="rec")
                        nc.vector.reciprocal_approx_fast(out=rec, in_=den)
                        sc = scpool.tile([1, IBLK], F32R, tag="sc", name="sc")
                        nc.vector.tensor_mul(
                            sc, rec, sgflat[0:1, S * h + IBLK * i:S * h + IBLK * (i + 1)])
                        bc = ps_bc.tile([128, IBLK], F32, tag="bc", name="bc")
                        nc.tensor.matmul(bc, onesr_sb, sc, start=True, stop=True)
                        bcs = scpool.tile([128, IBLK], F16, tag="bcs", name="bcs")
                        nc.scalar.copy(bcs, bc)
                        nc.vector.tensor_mul(OC[h][:, isl], psos[hh], bcs)
                # out-projection for this i-block's token tiles
                for t in range(4 * i, 4 * i + 4):
                    ysb = ypool.tile([128, HID], F16, tag="y", name="y")
                    for o in range(4):
                        psy = ps_y.tile([128, IBLK], F32, tag="psy", name="psy")
                        for cc in range(4):
                            nc.tensor.matmul(
                                psy, OC[cc][:, 128 * t:128 * (t + 1)],
                                wo_sb[cc][:, IBLK * o:IBLK * (o + 1)],
                                start=(cc == 0), stop=(cc == 3))
                        nc.scalar.copy(ysb[:, IBLK * o:IBLK * (o + 1)], psy)
                    nc.gpsimd.dma_start(
                        out=y_d[128 * t:128 * (t + 1), :], in_=ysb)

    nc.finalize()
    return nc


def kernel(hidden_states, cos, sin, w_qkv, w_o):
    global LAST_EXEC_NS, LAST_RESULTS
    from concourse.bass_utils import run_bass_kernel_spmd

    BF = np.float16
    hidden_states = np.asarray(hidden_states, dtype=np.float32)
    cos = np.asarray(cos, dtype=np.float32)
    sin = np.asarray(sin, dtype=np.float32)
    w_qkv = np.asarray(w_qkv, dtype=np.float32)
    w_o = np.asarray(w_o, dtype=np.float32)

    if "nc" not in _CACHE:
        _CACHE["nc"] = _build_program()
    nc = _CACHE["nc"]

    cosT = np.ascontiguousarray(cos.T)
    sinT = np.ascontiguousarray(sin.T)
    rotm = np.zeros((HD, HD), dtype=np.float32)
    for i in range(HD // 2):
        rotm[i + HD // 2, i] = -1.0   # rot[d'] = -q[d'+64] for d' < 64
        rotm[i, i + HD // 2] = 1.0    # rot[d'] = +q[d'-64] for d' >= 64
    rotm = rotm.astype(BF)
    ident = np.eye(128, dtype=np.float32).astype(BF)
    ones1 = np.ones((128, 1), dtype=np.float16)
    onesr = np.ones((1, 128), dtype=np.float32)

    xT = [np.ascontiguousarray(hidden_states[b].T).astype(BF) for b in range(B)]
    in_maps = []
    for c in range(N_CORES):
        b, g = divmod(c, TPG)
        qr = w_qkv[512 * g:512 * (g + 1)]
        kr = w_qkv[HID + GATE + 256 * g:HID + GATE + 256 * (g + 1)]
        vr = w_qkv[HID + GATE + KV_DIM + 256 * g:HID + GATE + KV_DIM + 256 * (g + 1)]
        gr = w_qkv[HID + QH * g:HID + QH * (g + 1)]
        pad = np.zeros((WCOL - 1024 - QH, HID), dtype=np.float32)
        wqkvT = np.ascontiguousarray(
            np.concatenate([qr, kr, vr, gr, pad], axis=0).T).astype(BF)
        woT = np.ascontiguousarray(w_o[:, 512 * g:512 * (g + 1)].T).astype(BF)
        in_maps.append({
            "xT": xT[b], "wqkvT": wqkvT, "woT": woT,
            "cosT": cosT, "sinT": sinT, "rotm": rotm, "ident": ident,
            "ones1": ones1, "onesr": onesr,
        })

    trace = bool(int(os.environ.get("KERNEL_TRACE", "0")))
    out = run_bass_kernel_spmd(nc, in_maps, list(range(N_CORES)), trace=trace)
    LAST_EXEC_NS = out.exec_time_ns
    LAST_RESULTS = out
    y = np.zeros((B, S, HID), dtype=np.float32)
    for c in range(N_CORES):
        b = c // TPG
        y[b] += np.asarray(out.results[c]["y"]).astype(np.float32)
    return y

